# revision 2
# baseline (speedup 1.0000x reference)
"""Causal self-attention (GQA, partial RoPE, qk rms-norm, logit softcap) on 8 trn2 cores.

Sharding: 8 cores = batch(2) x kv_head(4); host sums the 4 partial outputs per batch.

v2 design (vs baseline):
  - Wavefront emission: attention chunk c interleaves with qkv tiles 4c+4..4c+7
    (c<3) or with the output projection of chunks 0..2 (c==3), so ACT's tanh/exp
    stream overlaps PE's projection streams instead of serializing after them.
  - Attention processes g-PAIRS per kb: score tile [128, 2, 512-off] covers two
    q-heads at one kv block, with exact causal trimming (no garbage compute);
    diagonal blocks are zero-masked in-place by gpsimd affine_select (idle engine).
  - Softmax denominator: DVE pair-sums P2 = p(2i)+p(2i+1) (bf16 2x mode), then an
    all-ones-stationary matmul chain accumulates sum_sk and broadcasts it across
    all 128 partitions in one shot. Replaces 160 M=1 ones-matmuls + 16 [1,512]
    broadcasts (was ~32us of PE streams).
  - Sync-stall avoidance: micro-benchmarks showed PE matmuls pace at N/2.4+3ns
    (LDWEIGHTS fully hidden) unless a psum/pool recycle wait lands on them
    (+~110ns sem round trip). Pools are sized generously and filler MMs are
    emitted between dependent attention MMs; PSUM tags are split so long-lived
    tiles (ps_y, qkv accumulators) never share a rotation with transient ones.
  - DMA: host pre-lays x/wqkv/cos tables partition-contiguous so every transfer
    runs at full HBM bandwidth; weight chunks are issued before x so the first
    qkv matmul starts ~3us in (was ~25us).
"""
import math
import numpy as np
from contextlib import ExitStack

import ml_dtypes
import concourse.bass as bass
import concourse.tile as tile
from concourse import bacc, mybir
from concourse.bass_utils import run_bass_kernel_spmd
from concourse.masks import make_identity
from concourse.alu_op_type import AluOpType

F32 = mybir.dt.float32
BF16 = mybir.dt.bfloat16

B = 2
S = 2048
D = 2048
H = 16
HKV = 4
HD = 128
G = 4
ROPE = 32
HALF = ROPE // 2  # 16
ROPE_BASE = 10000.0
CAP = 30.0
EPS = float(np.finfo(np.float32).eps)
NST = S // 128   # 16
NCH = S // 512   # 4 chunks
NDT = D // 128   # 16
FQKV = G * HD + 2 * HD  # 768
NG5 = G + 1      # q heads + k processed as one 5-group row block

_CACHE = {}


def _build():
    nc = bacc.Bacc("TRN2", target_bir_lowering=False, debug=False)

    xch = nc.dram_tensor("xch", [128, NST, NDT, 128], BF16, kind="ExternalInput").ap()
    wqkv = nc.dram_tensor("wqkv", [128, NDT, FQKV], BF16, kind="ExternalInput").ap()
    wpT = nc.dram_tensor("wpT", [128, G, D], BF16, kind="ExternalInput").ap()
    gains = nc.dram_tensor("gains", [128, NG5], F32, kind="ExternalInput").ap()
    cos5 = nc.dram_tensor("cos5", [128, NST, NG5 * HALF], F32, kind="ExternalInput").ap()
    sin5 = nc.dram_tensor("sin5", [128, NST, NG5 * HALF], F32, kind="ExternalInput").ap()
    out = nc.dram_tensor("out", [S, D], BF16, kind="ExternalOutput").ap()
    out_r = out.rearrange("(t p) j -> t p j", p=128)

    with tile.TileContext(nc) as tc:
        with ExitStack() as ctx:
            persist = ctx.enter_context(tc.tile_pool(name="persist", bufs=1))

            ident = persist.tile([128, 128], BF16)
            make_identity(nc, ident)

            onesf = persist.tile([128, 128], F32)
            nc.vector.memset(onesf, 1.0)
            ones128 = persist.tile([128, 128], BF16)  # all-ones stationary: sum+bcast
            nc.vector.tensor_copy(ones128, onesf)

            eps_t = persist.tile([128, 1], F32)
            nc.vector.memset(eps_t, EPS)

            gains_sb = persist.tile([128, NG5], F32)
            gains4 = persist.tile([128, 4, NG5], F32)
            cos_all = persist.tile([128, NST, NG5 * HALF], F32)
            sin_all = persist.tile([128, NST, NG5 * HALF], F32)

            qT_all = persist.tile([128, G, S], BF16)    # [hd, g, sq]
            kT_all = persist.tile([128, S], BF16)       # [hd, sk]
            v_all = persist.tile([128, NST, HD], BF16)  # [sk%128, kb, hd]
            yT_all = persist.tile([128, G, S], BF16)    # [hd, g, sq]
            wq4 = [persist.tile([128, 4, FQKV], BF16, name=f"wq4_{_i}")
                   for _i in range(4)]
            wp2 = [persist.tile([128, 2, D], BF16, name=f"wp2_{_i}")
                   for _i in range(2)]
            ms_all = persist.tile([128, NST, NG5], F32)
            gsc_all = persist.tile([128, NST, NG5], F32)

            xc_pool = ctx.enter_context(tc.tile_pool(name="xc", bufs=4))
            sb = ctx.enter_context(tc.tile_pool(name="sb", bufs=2))
            # PSUM: acc = qkv combined tiles [128,768] / ps_y [128,2,512] / ptr
            # big = score pairs / D-accum / proj ps_o
            accq = ctx.enter_context(tc.tile_pool(name="accq", bufs=1, space="PSUM"))
            accy = ctx.enter_context(tc.tile_pool(name="accy", bufs=1, space="PSUM"))
            big = ctx.enter_context(tc.tile_pool(name="big", bufs=2, space="PSUM"))

            # ---- DMA prefetch (weights first; everything partition-contiguous)
            xcs = {}
            rots = {}

            def xc_dma(st):
                t = xc_pool.tile([128, NDT, 128], BF16, tag="xc", name="xcn")
                nc.sync.dma_start(out=t, in_=xch[:, st])
                xcs[st] = t

            xc_dma(0)
            nc.sync.dma_start(out=wq4[0], in_=wqkv[:, 0:4, :])
            xc_dma(1)
            nc.sync.dma_start(out=wq4[1], in_=wqkv[:, 4:8, :])
            xc_dma(2)
            nc.sync.dma_start(out=wq4[2], in_=wqkv[:, 8:12, :])
            xc_dma(3)
            nc.sync.dma_start(out=wq4[3], in_=wqkv[:, 12:16, :])
            nc.sync.dma_start(out=gains_sb, in_=gains)
            for _i4 in range(4):
                nc.vector.tensor_copy(gains4[:, _i4, :], gains_sb)
            nc.sync.dma_start(out=cos_all, in_=cos5)
            nc.sync.dma_start(out=sin_all, in_=sin5)

            # ---- qkv tile: combined q(4)|k|v projection, fused rope, transposes
            def qkv_gen(st):
                xc = xcs.pop(st)
                if st + 4 < NST:
                    xc_dma(st + 4)
                pq = accq.tile([128, FQKV], F32, tag="accq", name="pq")
                for dt in range(NDT):
                    wsl = wq4[dt // 4][:, dt % 4, :]
                    nc.tensor.matmul(pq[:, 0:512], xc[:, dt, :],
                                     wsl[:, 0:512],
                                     start=(dt == 0), stop=(dt == NDT - 1))
                    nc.tensor.matmul(pq[:, 512:768], xc[:, dt, :],
                                     wsl[:, 512:768],
                                     start=(dt == 0), stop=(dt == NDT - 1))
                    if dt % 4 == 3 and dt < NDT - 1:
                        yield
                # drain pq with 3 copies so its psum bufs free fast (the rope
                # chain then reads SBUF): v evac, rope-region, pass-through.
                pq5 = pq[:, 0:640].rearrange("p (g d) -> p g d", g=NG5)
                cos_t = cos_all[:, st, :].rearrange("p (g d) -> p g d", g=NG5)
                sin_t = sin_all[:, st, :].rearrange("p (g d) -> p g d", g=NG5)

                rot = sb.tile([128, NG5, HD], F32, tag="rot", bufs=6, name="rot")
                rin = sb.tile([128, NG5, ROPE], F32, tag="rin", name="rin")
                nc.vector.tensor_copy(v_all[:, st, :], pq[:, 640:768])
                nc.vector.tensor_copy(rin, pq5[:, :, 0:ROPE])
                nc.vector.tensor_copy(rot[:, :, ROPE:HD], pq5[:, :, ROPE:HD])

                tmp = sb.tile([128, NG5, HALF], F32, tag="tmp", name="tmp")
                nc.vector.tensor_mul(rot[:, :, 0:HALF], rin[:, :, 0:HALF], cos_t)
                nc.vector.tensor_mul(tmp, rin[:, :, HALF:ROPE], sin_t)
                nc.vector.tensor_add(rot[:, :, 0:HALF], rot[:, :, 0:HALF], tmp)
                nc.vector.tensor_mul(rot[:, :, HALF:ROPE], rin[:, :, HALF:ROPE], cos_t)
                nc.vector.tensor_mul(tmp, rin[:, :, 0:HALF], sin_t)
                nc.vector.tensor_sub(rot[:, :, HALF:ROPE], rot[:, :, HALF:ROPE], tmp)

                # rms stats after rope (rotation preserves the norm)
                sq = sb.tile([128, NG5 * HD], F32, tag="sq", name="sq")
                nc.vector.tensor_mul(sq, rot.rearrange("p g d -> p (g d)"),
                                     rot.rearrange("p g d -> p (g d)"))
                nc.vector.reduce_sum(ms_all[:, st, :],
                                     sq.rearrange("p (g d) -> p g d", g=NG5),
                                     axis=mybir.AxisListType.X)
                rots[st] = rot
                yield

            def rstd_batch(sts):
                # one ACT Rsqrt for 4 s-tiles: keeps the exp/tanh activation
                # table resident during attention (Sqrt lives in another set)
                rstd = sb.tile([128, len(sts), NG5], F32, tag="rstd", name="rstd")
                nc.scalar.activation(rstd, ms_all[:, sts[0]:sts[0] + len(sts), :],
                                     mybir.ActivationFunctionType.Sqrt,
                                     scale=1.0 / HD, bias=eps_t)
                nc.vector.reciprocal(rstd, rstd)
                nc.vector.tensor_mul(gsc_all[:, sts[0]:sts[0] + len(sts), :],
                                     rstd, gains4[:, 0:len(sts), :])

            def qkv_fin(st):
                rot = rots.pop(st)
                qk = sb.tile([128, NG5, HD], BF16, tag="qk", name="qk")
                for h in range(NG5):
                    nc.vector.tensor_scalar_mul(qk[:, h, :], rot[:, h, :],
                                                gsc_all[:, st, h:h + 1])
                # 5 PE transposes into a psum tile, DVE evacuation
                ptr = big.tile([128, NG5 * HD], BF16, tag="big", name="ptr")
                for h in range(NG5):
                    nc.tensor.transpose(ptr[:, h * HD:(h + 1) * HD], qk[:, h, :], ident)
                nc.vector.tensor_copy(
                    qT_all[:, :, st * 128:(st + 1) * 128],
                    ptr[:, 0:4 * HD].rearrange("p (g d) -> p g d", g=G))
                nc.vector.tensor_copy(kT_all[:, st * 128:(st + 1) * 128],
                                      ptr[:, 4 * HD:5 * HD])

            # ---- attention: chunk c, pass gp covers heads (2gp, 2gp+1)
            def attn_gen(c):
                nkv = 4 * (c + 1)
                for gp in range(2):
                    g0 = 2 * gp
                    qT_c = qT_all[:, g0:g0 + 2, c * 512:(c + 1) * 512]
                    ps_y = accy.tile([128, 2, 512], F32, tag="accy", name="ps_y")
                    p2s = []
                    prev_p = None
                    for kb in range(nkv):
                        r = kb - 4 * c
                        off = 128 * r if r > 0 else 0
                        w = 512 - off
                        ps_s = big.tile([128, 2, 512], F32, tag="big", name="ps_s")
                        for i in range(2):
                            nc.tensor.matmul(ps_s[:, i, off:512],
                                             kT_all[:, kb * 128:(kb + 1) * 128],
                                             qT_c[:, i, off:512],
                                             start=True, stop=True)
                        t = sb.tile([128, 2, 512], F32, tag="t", bufs=4, name="t")
                        nc.scalar.activation(t[:, :, off:512], ps_s[:, :, off:512],
                                             mybir.ActivationFunctionType.Tanh,
                                             scale=1.0 / CAP)
                        p = sb.tile([128, 2, 512], BF16, tag="p", bufs=8, name="p")
                        nc.scalar.activation(p[:, :, off:512], t[:, :, off:512],
                                             mybir.ActivationFunctionType.Exp,
                                             scale=CAP)
                        if r >= 0:
                            # zero everything below the diagonal (keep
                            # sq >= sk + 128r) over the FULL width: columns
                            # [0:off] hold stale pool garbage that the pair-sum
                            # would otherwise pick up, and fill overwrites them.
                            nc.gpsimd.affine_select(
                                out=p, in_=p,
                                compare_op=AluOpType.is_ge, fill=0.0,
                                base=-128 * r, pattern=[[0, 2], [1, 512]],
                                channel_multiplier=-1)
                        for i in range(2):
                            nc.tensor.matmul(ps_y[:, i, off:512],
                                             v_all[:, kb, :], p[:, i, off:512],
                                             start=(kb == 0), stop=(kb == nkv - 1))
                        if kb % 2 == 1:
                            # denominator pair-sum (bf16 2x); off of the even kb
                            poff = 128 * (kb - 1 - 4 * c) if kb - 1 - 4 * c > 0 else 0
                            p2 = sb.tile([128, 2, 512], BF16, tag="p2", bufs=10,
                                         name="p2")
                            nc.vector.tensor_add(p2[:, :, poff:512],
                                                 prev_p[:, :, poff:512],
                                                 p[:, :, poff:512])
                            p2s.append((p2, poff))
                        prev_p = p
                        yield
                    # pass end: sum over sk + broadcast via all-ones stationary
                    ps_d = big.tile([128, 2, 512], F32, tag="big", name="ps_d")
                    npair = len(p2s)
                    for i in range(2):
                        for j, (p2, poff) in enumerate(p2s):
                            nc.tensor.matmul(ps_d[:, i, poff:512], ones128,
                                             p2[:, i, poff:512],
                                             start=(j == 0), stop=(j == npair - 1))
                    recip = sb.tile([128, 2, 512], F32, tag="recip", name="recip")
                    nc.vector.reciprocal_approx_fast(out=recip, in_=ps_d)
                    nc.vector.tensor_mul(
                        yT_all[:, g0:g0 + 2, c * 512:(c + 1) * 512], ps_y, recip)
                    yield

            # ---- output projection group (ps_o alternates between the freed
            # qkv psum tag and the big tag so scores keep their double-buffer)
            proj_ctr = [0]

            def proj_group(st, jc, tail=False):
                proj_ctr[0] += 1
                if proj_ctr[0] % 2 == 0:
                    ps_o = accq.tile([128, 512], F32, tag="accq", name="ps_o")
                else:
                    ps_o = big.tile([128, 512], F32, tag="big", name="ps_o")
                for g in range(G):
                    nc.tensor.matmul(ps_o,
                                     yT_all[:, g, st * 128:(st + 1) * 128],
                                     wp2[g // 2][:, g % 2, jc * 512:(jc + 1) * 512],
                                     start=(g == 0), stop=(g == G - 1))
                o_sb = sb.tile([128, 512], BF16, tag="o_sb", bufs=6, name="o_sb")
                if tail:
                    nc.scalar.copy(o_sb, ps_o)  # ACT is idle once attention ends
                else:
                    nc.vector.tensor_copy(o_sb, ps_o)
                nc.sync.dma_start(out=out_r[st][:, jc * 512:(jc + 1) * 512], in_=o_sb)

            # ---- wavefront driver
            def drain(gen):
                for _ in gen:
                    pass

            # startup: qkv st0-4; windows are skewed by one tile so the
            # rstd/transpose chain of chunk c+1 always completes mid-window
            drain(qkv_gen(0))
            drain(qkv_gen(1))
            rstd_batch([0, 1])
            qkv_fin(0)
            qkv_fin(1)
            drain(qkv_gen(2))
            drain(qkv_gen(3))
            nc.sync.dma_start(out=wp2[0], in_=wpT[:, 0:2, :])
            nc.sync.dma_start(out=wp2[1], in_=wpT[:, 2:4, :])
            rstd_batch([2, 3])
            qkv_fin(2)
            qkv_fin(3)
            drain(qkv_gen(4))

            pending = []
            for c in range(4):
                attn = attn_gen(c)
                qsts = [st for st in range(4 * c + 5, 4 * c + 9) if st < NST]
                fsteps = [qkv_gen(st) for st in qsts]
                if c == 3:
                    pending += [(st, jc) for st in range(0, 12) for jc in range(4)]
                done_a = False
                fi = 0
                pb = 0.0
                while not done_a or fi < len(fsteps) or (done_a and pending and c == 3 and pb < 990.0):
                    if not done_a:
                        try:
                            next(attn)
                        except StopIteration:
                            done_a = True
                    if fi < len(fsteps):
                        try:
                            next(fsteps[fi])
                        except StopIteration:
                            fi += 1
                            if fi == 3:
                                rstd_batch(list(range(4 * c + 4, 4 * c + 8)))
                                for _s in range(4 * c + 4, 4 * c + 8):
                                    qkv_fin(_s)
                    elif pending:
                        if done_a:
                            pb = 999.0
                            break
                        pb += 1.3
                        while pb >= 1.0 and pending:
                            proj_group(*pending.pop(0))
                            pb -= 1.0
            while pending:
                proj_group(*pending.pop(0))
            for st in range(12, 16):
                for jc in range(4):
                    proj_group(st, jc, tail=True)

    nc.compile()
    return nc


def _host_prep(x, Wq, Wk, Wv, Wproj, q_gain):
    inv_freq = 1.0 / (ROPE_BASE ** (np.arange(0, ROPE, 2, dtype=np.float32) / ROPE))
    t = np.arange(S, dtype=np.float32)
    freqs = np.outer(t, inv_freq).astype(np.float32)  # [S, 16]
    cos = np.cos(freqs).astype(np.float32)
    sin = np.sin(freqs).astype(np.float32)
    # [S, 5*16] tables (4 q heads + k), then partition-contiguous [128, NST, 80]
    cos5 = np.tile(cos[:, None, :], (1, NG5, 1)).reshape(S, NG5 * HALF)
    sin5 = np.tile(sin[:, None, :], (1, NG5, 1)).reshape(S, NG5 * HALF)
    cos5 = np.ascontiguousarray(cos5.reshape(NST, 128, NG5 * HALF).transpose(1, 0, 2))
    sin5 = np.ascontiguousarray(sin5.reshape(NST, 128, NG5 * HALF).transpose(1, 0, 2))

    # x^T chunks, partition-contiguous: xch[p, st, dt, s] = x[b][st*128+s, dt*128+p]
    xchs = []
    for b in range(B):
        xT = x[b].T.astype(ml_dtypes.bfloat16)          # [D, S]
        xc = xT.reshape(NDT, 128, NST, 128)             # [dt, p, st, s]
        xchs.append(np.ascontiguousarray(xc.transpose(1, 2, 0, 3)))

    in_maps = []
    for core in range(8):
        b, h = core // HKV, core % HKV
        wq = np.concatenate(
            [Wq[512 * h:512 * h + 512].T,
             Wk[128 * h:128 * h + 128].T,
             Wv[128 * h:128 * h + 128].T], axis=1).astype(ml_dtypes.bfloat16)
        # [D, 768] -> [128, NDT, 768] partition-contiguous
        wq = np.ascontiguousarray(wq.reshape(NDT, 128, FQKV).transpose(1, 0, 2))
        wp = Wproj[:, 512 * h:512 * h + 512].T.astype(ml_dtypes.bfloat16)  # [512, D]
        wp = np.ascontiguousarray(wp.reshape(G, 128, D).transpose(1, 0, 2))
        gains5 = np.concatenate(
            [(q_gain[G * h:G * h + G] / math.sqrt(HD)).astype(np.float32),
             np.ones(1, np.float32)])
        gains5 = np.ascontiguousarray(np.broadcast_to(gains5[None, :], (128, NG5)))
        in_maps.append({
            "xch": xchs[b],
            "wqkv": wq,
            "wpT": wp,
            "gains": gains5,
            "cos5": cos5,
            "sin5": sin5,
        })
    return in_maps


def kernel(x, Wq, Wk, Wv, Wproj, q_gain, _trace=False):
    x = np.asarray(x, dtype=np.float32)
    Wq = np.asarray(Wq, dtype=np.float32)
    Wk = np.asarray(Wk, dtype=np.float32)
    Wv = np.asarray(Wv, dtype=np.float32)
    Wproj = np.asarray(Wproj, dtype=np.float32)
    q_gain = np.asarray(q_gain, dtype=np.float32)

    if "nc" not in _CACHE:
        _CACHE["nc"] = _build()
    nc = _CACHE["nc"]

    in_maps = _host_prep(x, Wq, Wk, Wv, Wproj, q_gain)
    res = run_bass_kernel_spmd(nc, in_maps, core_ids=list(range(8)), trace=_trace)

    out = np.empty((B, S, D), dtype=np.float32)
    for b in range(B):
        acc = np.zeros((S, D), dtype=np.float64)
        for h in range(HKV):
            acc += res.results[b * HKV + h]["out"]
        out[b] = acc.astype(np.float32)
    if _trace:
        return out, res
    return out


# revision 3
# speedup vs baseline: 1.0073x; 1.0073x over previous
"""Causal self-attention (GQA, partial RoPE, qk rms-norm, logit softcap) on 8 trn2 cores.

Sharding: 8 cores = batch(2) x kv_head(4); host sums the 4 partial outputs per batch.

v2 design (vs baseline):
  - Wavefront emission: attention chunk c interleaves with qkv tiles 4c+4..4c+7
    (c<3) or with the output projection of chunks 0..2 (c==3), so ACT's tanh/exp
    stream overlaps PE's projection streams instead of serializing after them.
  - Attention processes g-PAIRS per kb: score tile [128, 2, 512-off] covers two
    q-heads at one kv block, with exact causal trimming (no garbage compute);
    diagonal blocks are zero-masked in-place by gpsimd affine_select (idle engine).
  - Softmax denominator: DVE pair-sums P2 = p(2i)+p(2i+1) (bf16 2x mode), then an
    all-ones-stationary matmul chain accumulates sum_sk and broadcasts it across
    all 128 partitions in one shot. Replaces 160 M=1 ones-matmuls + 16 [1,512]
    broadcasts (was ~32us of PE streams).
  - Sync-stall avoidance: micro-benchmarks showed PE matmuls pace at N/2.4+3ns
    (LDWEIGHTS fully hidden) unless a psum/pool recycle wait lands on them
    (+~110ns sem round trip). Pools are sized generously and filler MMs are
    emitted between dependent attention MMs; PSUM tags are split so long-lived
    tiles (ps_y, qkv accumulators) never share a rotation with transient ones.
  - DMA: host pre-lays x/wqkv/cos tables partition-contiguous so every transfer
    runs at full HBM bandwidth; weight chunks are issued before x so the first
    qkv matmul starts ~3us in (was ~25us).
"""
import math
import numpy as np
from contextlib import ExitStack

import ml_dtypes
import concourse.bass as bass
import concourse.tile as tile
from concourse import bacc, mybir
from concourse.bass_utils import run_bass_kernel_spmd
from concourse.masks import make_identity
from concourse.alu_op_type import AluOpType

F32 = mybir.dt.float32
BF16 = mybir.dt.bfloat16

B = 2
S = 2048
D = 2048
H = 16
HKV = 4
HD = 128
G = 4
ROPE = 32
HALF = ROPE // 2  # 16
ROPE_BASE = 10000.0
CAP = 30.0
EPS = float(np.finfo(np.float32).eps)
NST = S // 128   # 16
NCH = S // 512   # 4 chunks
NDT = D // 128   # 16
FQKV = G * HD + 2 * HD  # 768
NG5 = G + 1      # q heads + k processed as one 5-group row block

_CACHE = {}


def _build():
    nc = bacc.Bacc("TRN2", target_bir_lowering=False, debug=False)

    xch = nc.dram_tensor("xch", [128, NST, NDT, 128], BF16, kind="ExternalInput").ap()
    wqkv = nc.dram_tensor("wqkv", [128, NDT, FQKV], BF16, kind="ExternalInput").ap()
    wpT = nc.dram_tensor("wpT", [128, G, D], BF16, kind="ExternalInput").ap()
    gains = nc.dram_tensor("gains", [128, NG5], F32, kind="ExternalInput").ap()
    cos5 = nc.dram_tensor("cos5", [128, NST, NG5 * HALF], F32, kind="ExternalInput").ap()
    sin5 = nc.dram_tensor("sin5", [128, NST, NG5 * HALF], F32, kind="ExternalInput").ap()
    out = nc.dram_tensor("out", [S, D], BF16, kind="ExternalOutput").ap()
    out_r = out.rearrange("(t p) j -> t p j", p=128)

    with tile.TileContext(nc) as tc:
        with ExitStack() as ctx:
            persist = ctx.enter_context(tc.tile_pool(name="persist", bufs=1))

            ident = persist.tile([128, 128], BF16)
            make_identity(nc, ident)

            onesf = persist.tile([128, 128], F32)
            nc.vector.memset(onesf, 1.0)
            ones128 = persist.tile([128, 128], BF16)  # all-ones stationary: sum+bcast
            nc.vector.tensor_copy(ones128, onesf)

            eps_t = persist.tile([128, 1], F32)
            nc.vector.memset(eps_t, EPS)


            gains_sb = persist.tile([128, NG5], F32)
            gains4 = persist.tile([128, 4, NG5], F32)
            cos_all = persist.tile([128, NST, NG5 * HALF], F32)
            sin_all = persist.tile([128, NST, NG5 * HALF], F32)

            qT_all = persist.tile([128, G, S], BF16)    # [hd, g, sq]
            kT_all = persist.tile([128, S], BF16)       # [hd, sk]
            v_all = persist.tile([128, NST, HD], BF16)  # [sk%128, kb, hd]
            yT_all = persist.tile([128, G, S], BF16)    # [hd, g, sq]
            wq4 = [persist.tile([128, 4, FQKV], BF16, name=f"wq4_{_i}")
                   for _i in range(4)]
            wp2 = [persist.tile([128, 2, D], BF16, name=f"wp2_{_i}")
                   for _i in range(2)]
            ms_all = persist.tile([128, NST, NG5], F32)
            gsc_all = persist.tile([128, NST, NG5], F32)

            xc_pool = ctx.enter_context(tc.tile_pool(name="xc", bufs=4))
            sb = ctx.enter_context(tc.tile_pool(name="sb", bufs=2))
            # PSUM: acc = qkv combined tiles [128,768] / ps_y [128,2,512] / ptr
            # big = score pairs / D-accum / proj ps_o
            accq = ctx.enter_context(tc.tile_pool(name="accq", bufs=1, space="PSUM"))
            accy = ctx.enter_context(tc.tile_pool(name="accy", bufs=1, space="PSUM"))
            big = ctx.enter_context(tc.tile_pool(name="big", bufs=2, space="PSUM"))

            # HAM warmup: dummy matmuls with no DMA deps fill the initial
            # weight-DMA wait and bring the PE clock to 2.4GHz before the
            # first real matmul. The tile is written, never read; its buf
            # frees on write-completion long before attention needs it.
            warm = big.tile([128, 2, 512], F32, tag="big", name="warm")
            for _w in range(64):
                nc.tensor.matmul(warm[:, 0, 0:128], ident, ident,
                                 start=True, stop=True)

            # ---- DMA prefetch (weights first; everything partition-contiguous)
            xcs = {}
            rots = {}

            def xc_dma(st):
                t = xc_pool.tile([128, NDT, 128], BF16, tag="xc", name="xcn")
                nc.sync.dma_start(out=t, in_=xch[:, st])
                xcs[st] = t

            xc_dma(0)
            nc.sync.dma_start(out=wq4[0], in_=wqkv[:, 0:4, :])
            xc_dma(1)
            nc.sync.dma_start(out=wq4[1], in_=wqkv[:, 4:8, :])
            xc_dma(2)
            nc.sync.dma_start(out=wq4[2], in_=wqkv[:, 8:12, :])
            xc_dma(3)
            nc.sync.dma_start(out=wq4[3], in_=wqkv[:, 12:16, :])
            nc.sync.dma_start(out=gains_sb, in_=gains)
            for _i4 in range(4):
                nc.vector.tensor_copy(gains4[:, _i4, :], gains_sb)
            nc.sync.dma_start(out=cos_all, in_=cos5)
            nc.sync.dma_start(out=sin_all, in_=sin5)

            # ---- qkv tile: combined q(4)|k|v projection, fused rope, transposes
            def qkv_gen(st):
                xc = xcs.pop(st)
                if st + 4 < NST:
                    xc_dma(st + 4)
                pq = accq.tile([128, FQKV], F32, tag="accq", name="pq")
                for dt in range(NDT):
                    wsl = wq4[dt // 4][:, dt % 4, :]
                    nc.tensor.matmul(pq[:, 0:512], xc[:, dt, :],
                                     wsl[:, 0:512],
                                     start=(dt == 0), stop=(dt == NDT - 1))
                    nc.tensor.matmul(pq[:, 512:768], xc[:, dt, :],
                                     wsl[:, 512:768],
                                     start=(dt == 0), stop=(dt == NDT - 1))
                    if dt % 4 == 3 and dt < NDT - 1:
                        yield
                # drain pq with 3 copies so its psum bufs free fast (the rope
                # chain then reads SBUF): v evac, rope-region, pass-through.
                pq5 = pq[:, 0:640].rearrange("p (g d) -> p g d", g=NG5)
                cos_t = cos_all[:, st, :].rearrange("p (g d) -> p g d", g=NG5)
                sin_t = sin_all[:, st, :].rearrange("p (g d) -> p g d", g=NG5)

                rot = sb.tile([128, NG5, HD], F32, tag="rot", bufs=6, name="rot")
                rin = sb.tile([128, NG5, ROPE], F32, tag="rin", name="rin")
                # high priority: these three copies free the qkv psum buf the
                # next s-tile's matmuls are waiting on — jump the DVE queue
                with tc.high_priority(offset=60):
                    nc.vector.tensor_copy(v_all[:, st, :], pq[:, 640:768])
                    nc.vector.tensor_copy(rin, pq5[:, :, 0:ROPE])
                    nc.vector.tensor_copy(rot[:, :, ROPE:HD], pq5[:, :, ROPE:HD])

                tmp = sb.tile([128, NG5, HALF], F32, tag="tmp", name="tmp")
                nc.vector.tensor_mul(rot[:, :, 0:HALF], rin[:, :, 0:HALF], cos_t)
                nc.vector.tensor_mul(tmp, rin[:, :, HALF:ROPE], sin_t)
                nc.vector.tensor_add(rot[:, :, 0:HALF], rot[:, :, 0:HALF], tmp)
                nc.vector.tensor_mul(rot[:, :, HALF:ROPE], rin[:, :, HALF:ROPE], cos_t)
                nc.vector.tensor_mul(tmp, rin[:, :, 0:HALF], sin_t)
                nc.vector.tensor_sub(rot[:, :, HALF:ROPE], rot[:, :, HALF:ROPE], tmp)

                # rms stats after rope (rotation preserves the norm)
                sq = sb.tile([128, NG5 * HD], F32, tag="sq", name="sq")
                nc.vector.tensor_mul(sq, rot.rearrange("p g d -> p (g d)"),
                                     rot.rearrange("p g d -> p (g d)"))
                nc.vector.reduce_sum(ms_all[:, st, :],
                                     sq.rearrange("p (g d) -> p g d", g=NG5),
                                     axis=mybir.AxisListType.X)
                rots[st] = rot
                yield

            def rstd_batch(sts, _hp=True):
                # one ACT Rsqrt for 4 s-tiles: keeps the exp/tanh activation
                # table resident during attention (Sqrt lives in another set)
                rstd = sb.tile([128, len(sts), NG5], F32, tag="rstd", name="rstd")
                with tc.high_priority(offset=80):
                    nc.scalar.activation(rstd, ms_all[:, sts[0]:sts[0] + len(sts), :],
                                         mybir.ActivationFunctionType.Sqrt,
                                         scale=1.0 / HD, bias=eps_t)
                    nc.vector.reciprocal(rstd, rstd)
                    nc.vector.tensor_mul(gsc_all[:, sts[0]:sts[0] + len(sts), :],
                                         rstd, gains4[:, 0:len(sts), :])

            def qkv_fin(st):
                rot = rots.pop(st)
                qk = sb.tile([128, NG5, HD], BF16, tag="qk", name="qk")
                with tc.high_priority(offset=80):
                    for h in range(NG5):
                        nc.vector.tensor_scalar_mul(qk[:, h, :], rot[:, h, :],
                                                    gsc_all[:, st, h:h + 1])
                # 5 PE transposes into a psum tile, DVE evacuation
                ptr = big.tile([128, NG5 * HD], BF16, tag="big", name="ptr")
                for h in range(NG5):
                    nc.tensor.transpose(ptr[:, h * HD:(h + 1) * HD], qk[:, h, :], ident)
                nc.vector.tensor_copy(
                    qT_all[:, :, st * 128:(st + 1) * 128],
                    ptr[:, 0:4 * HD].rearrange("p (g d) -> p g d", g=G))
                nc.vector.tensor_copy(kT_all[:, st * 128:(st + 1) * 128],
                                      ptr[:, 4 * HD:5 * HD])

            # ---- attention: chunk c, pass gp covers heads (2gp, 2gp+1)
            def attn_gen(c):
                nkv = 4 * (c + 1)
                for gp in range(2):
                    g0 = 2 * gp
                    qT_c = qT_all[:, g0:g0 + 2, c * 512:(c + 1) * 512]
                    ps_y = accy.tile([128, 2, 512], F32, tag="accy", name="ps_y")
                    p2s = []
                    prev_p = None
                    for kb in range(nkv):
                        r = kb - 4 * c
                        off = 128 * r if r > 0 else 0
                        w = 512 - off
                        ps_s = big.tile([128, 2, 512], F32, tag="big", name="ps_s")
                        for i in range(2):
                            nc.tensor.matmul(ps_s[:, i, off:512],
                                             kT_all[:, kb * 128:(kb + 1) * 128],
                                             qT_c[:, i, off:512],
                                             start=True, stop=True)
                        t = sb.tile([128, 2, 512], F32, tag="t", bufs=4, name="t")
                        nc.scalar.activation(t[:, :, off:512], ps_s[:, :, off:512],
                                             mybir.ActivationFunctionType.Tanh,
                                             scale=1.0 / CAP)
                        p = sb.tile([128, 2, 512], BF16, tag="p", bufs=10, name="p")
                        nc.scalar.activation(p[:, :, off:512], t[:, :, off:512],
                                             mybir.ActivationFunctionType.Exp,
                                             scale=CAP)
                        if r >= 0:
                            # zero everything below the diagonal (keep
                            # sq >= sk + 128r) over the FULL width: columns
                            # [0:off] hold stale pool garbage that the pair-sum
                            # would otherwise pick up, and fill overwrites them.
                            nc.gpsimd.affine_select(
                                out=p, in_=p,
                                compare_op=AluOpType.is_ge, fill=0.0,
                                base=-128 * r, pattern=[[0, 2], [1, 512]],
                                channel_multiplier=-1)
                        for i in range(2):
                            nc.tensor.matmul(ps_y[:, i, off:512],
                                             v_all[:, kb, :], p[:, i, off:512],
                                             start=(kb == 0), stop=(kb == nkv - 1))
                        if kb % 2 == 1:
                            # denominator pair-sum (bf16 2x); off of the even kb
                            poff = 128 * (kb - 1 - 4 * c) if kb - 1 - 4 * c > 0 else 0
                            p2 = sb.tile([128, 2, 512], BF16, tag="p2", bufs=10,
                                         name="p2")
                            nc.vector.tensor_add(p2[:, :, poff:512],
                                                 prev_p[:, :, poff:512],
                                                 p[:, :, poff:512])
                            p2s.append((p2, poff))
                        prev_p = p
                        yield
                    # pass end: sum over sk + broadcast via all-ones stationary
                    ps_d = big.tile([128, 2, 512], F32, tag="big", name="ps_d")
                    npair = len(p2s)
                    for i in range(2):
                        for j, (p2, poff) in enumerate(p2s):
                            nc.tensor.matmul(ps_d[:, i, poff:512], ones128,
                                             p2[:, i, poff:512],
                                             start=(j == 0), stop=(j == npair - 1))
                    recip = sb.tile([128, 2, 512], F32, tag="recip", name="recip")
                    nc.vector.reciprocal_approx_fast(out=recip, in_=ps_d)
                    nc.vector.tensor_mul(
                        yT_all[:, g0:g0 + 2, c * 512:(c + 1) * 512], ps_y, recip)
                    yield

            # ---- output projection group (ps_o alternates between the freed
            # qkv psum tag and the big tag so scores keep their double-buffer)
            proj_ctr = [0]

            def proj_group(st, jc, tail=False):
                proj_ctr[0] += 1
                if tail and proj_ctr[0] % 3 == 0:
                    # attention is done: the ps_y bank pair is free too
                    ps_o = accy.tile([128, 512], F32, tag="accy", name="ps_o")
                elif proj_ctr[0] % 2 == 0:
                    ps_o = accq.tile([128, 512], F32, tag="accq", name="ps_o")
                else:
                    ps_o = big.tile([128, 512], F32, tag="big", name="ps_o")
                for g in range(G):
                    nc.tensor.matmul(ps_o,
                                     yT_all[:, g, st * 128:(st + 1) * 128],
                                     wp2[g // 2][:, g % 2, jc * 512:(jc + 1) * 512],
                                     start=(g == 0), stop=(g == G - 1))
                o_sb = sb.tile([128, 512], BF16, tag="o_sb", bufs=6, name="o_sb")
                if tail:
                    nc.scalar.copy(o_sb, ps_o)  # ACT is idle once attention ends
                else:
                    nc.vector.tensor_copy(o_sb, ps_o)
                nc.sync.dma_start(out=out_r[st][:, jc * 512:(jc + 1) * 512], in_=o_sb)

            # ---- wavefront driver
            def drain(gen):
                for _ in gen:
                    pass

            # startup: qkv st0-4; windows are skewed by one tile so the
            # rstd/transpose chain of chunk c+1 always completes mid-window
            drain(qkv_gen(0))
            drain(qkv_gen(1))
            rstd_batch([0, 1])
            qkv_fin(0)
            qkv_fin(1)
            drain(qkv_gen(2))
            drain(qkv_gen(3))
            nc.sync.dma_start(out=wp2[0], in_=wpT[:, 0:2, :])
            nc.sync.dma_start(out=wp2[1], in_=wpT[:, 2:4, :])
            rstd_batch([2, 3])
            qkv_fin(2)
            qkv_fin(3)
            drain(qkv_gen(4))

            pending = []
            for c in range(4):
                attn = attn_gen(c)
                qsts = [st for st in range(4 * c + 5, 4 * c + 9) if st < NST]
                fsteps = [qkv_gen(st) for st in qsts]
                if c == 3:
                    pending += [(st, jc) for st in range(0, 12) for jc in range(4)]
                done_a = False
                fi = 0
                pb = 0.0
                while not done_a or fi < len(fsteps) or (done_a and pending and c == 3 and pb < 990.0):
                    if not done_a:
                        try:
                            next(attn)
                        except StopIteration:
                            done_a = True
                    if fi < len(fsteps):
                        try:
                            next(fsteps[fi])
                        except StopIteration:
                            fi += 1
                            if fi == 3:
                                rstd_batch(list(range(4 * c + 4, 4 * c + 8)))
                                for _s in range(4 * c + 4, 4 * c + 8):
                                    qkv_fin(_s)
                    elif pending:
                        if done_a:
                            pb = 999.0
                            break
                        pb += 1.3
                        while pb >= 1.0 and pending:
                            proj_group(*pending.pop(0))
                            pb -= 1.0
            while pending:
                proj_group(*pending.pop(0))
            for st in range(12, 16):
                for jc in range(4):
                    proj_group(st, jc, tail=True)

    nc.compile()
    return nc


def _host_prep(x, Wq, Wk, Wv, Wproj, q_gain):
    inv_freq = 1.0 / (ROPE_BASE ** (np.arange(0, ROPE, 2, dtype=np.float32) / ROPE))
    t = np.arange(S, dtype=np.float32)
    freqs = np.outer(t, inv_freq).astype(np.float32)  # [S, 16]
    cos = np.cos(freqs).astype(np.float32)
    sin = np.sin(freqs).astype(np.float32)
    # [S, 5*16] tables (4 q heads + k), then partition-contiguous [128, NST, 80]
    cos5 = np.tile(cos[:, None, :], (1, NG5, 1)).reshape(S, NG5 * HALF)
    sin5 = np.tile(sin[:, None, :], (1, NG5, 1)).reshape(S, NG5 * HALF)
    cos5 = np.ascontiguousarray(cos5.reshape(NST, 128, NG5 * HALF).transpose(1, 0, 2))
    sin5 = np.ascontiguousarray(sin5.reshape(NST, 128, NG5 * HALF).transpose(1, 0, 2))

    # x^T chunks, partition-contiguous: xch[p, st, dt, s] = x[b][st*128+s, dt*128+p]
    xchs = []
    for b in range(B):
        xT = x[b].T.astype(ml_dtypes.bfloat16)          # [D, S]
        xc = xT.reshape(NDT, 128, NST, 128)             # [dt, p, st, s]
        xchs.append(np.ascontiguousarray(xc.transpose(1, 2, 0, 3)))

    in_maps = []
    for core in range(8):
        b, h = core // HKV, core % HKV
        wq = np.concatenate(
            [Wq[512 * h:512 * h + 512].T,
             Wk[128 * h:128 * h + 128].T,
             Wv[128 * h:128 * h + 128].T], axis=1).astype(ml_dtypes.bfloat16)
        # [D, 768] -> [128, NDT, 768] partition-contiguous
        wq = np.ascontiguousarray(wq.reshape(NDT, 128, FQKV).transpose(1, 0, 2))
        wp = Wproj[:, 512 * h:512 * h + 512].T.astype(ml_dtypes.bfloat16)  # [512, D]
        wp = np.ascontiguousarray(wp.reshape(G, 128, D).transpose(1, 0, 2))
        gains5 = np.concatenate(
            [(q_gain[G * h:G * h + G] / math.sqrt(HD)).astype(np.float32),
             np.ones(1, np.float32)])
        gains5 = np.ascontiguousarray(np.broadcast_to(gains5[None, :], (128, NG5)))
        in_maps.append({
            "xch": xchs[b],
            "wqkv": wq,
            "wpT": wp,
            "gains": gains5,
            "cos5": cos5,
            "sin5": sin5,
        })
    return in_maps


def kernel(x, Wq, Wk, Wv, Wproj, q_gain, _trace=False):
    x = np.asarray(x, dtype=np.float32)
    Wq = np.asarray(Wq, dtype=np.float32)
    Wk = np.asarray(Wk, dtype=np.float32)
    Wv = np.asarray(Wv, dtype=np.float32)
    Wproj = np.asarray(Wproj, dtype=np.float32)
    q_gain = np.asarray(q_gain, dtype=np.float32)

    if "nc" not in _CACHE:
        _CACHE["nc"] = _build()
    nc = _CACHE["nc"]

    in_maps = _host_prep(x, Wq, Wk, Wv, Wproj, q_gain)
    res = run_bass_kernel_spmd(nc, in_maps, core_ids=list(range(8)), trace=_trace)

    out = np.empty((B, S, D), dtype=np.float32)
    for b in range(B):
        acc = np.zeros((S, D), dtype=np.float64)
        for h in range(HKV):
            acc += res.results[b * HKV + h]["out"]
        out[b] = acc.astype(np.float32)
    if _trace:
        return out, res
    return out
